# revision 18
# baseline (speedup 1.0000x reference)
# GATv2 encoder (3x GATv2Conv, H=1) on 8 Trainium2 NeuronCores.
#
# Sharding: nodes partitioned by dst across 8 cores (graph parallel).
# Edge work per core is organized as 98 "bins" of <=128 dst nodes each
# (host-side bin-packing balances edge counts), each bin = T tiles of 128
# edges.  Per tile: indirect-DMA gather of source features, leaky-relu
# attention logits, exp, and a one-hot matmul on the TensorEngine that
# accumulates both the softmax numerator and denominator in PSUM.
# Host gathers the per-core dense outputs between the three launches
# (all-gather of the xl tables).
import os
import sys
import math
import functools
import numpy as np

for _p in ("/opt/trn_rl_repo",):
    if _p not in sys.path and os.path.isdir(_p):
        sys.path.insert(0, _p)

import concourse.bass as bass
import concourse.mybir as mybir
import concourse.tile as tile
from concourse import bacc
from concourse.bass import IndirectOffsetOnAxis

F32 = mybir.dt.float32
I32 = mybir.dt.int32
AF = mybir.ActivationFunctionType
ALU = mybir.AluOpType

# Problem constants (hardcoded per contract)
N = 100_000
E = 1_600_000
IN, HID, OUT, H = 256, 128, 64, 1
SLOPE = 0.2
NCORES = 8
P = 128
EPS = 1e-30


class Cfg:
    """Geometry, parameterized so small test instances can be built."""

    def __init__(self, n=N, e=E, fin=IN, hid=HID, out=OUT, ncores=NCORES):
        self.n, self.e, self.fin, self.hid, self.out = n, e, fin, hid, out
        self.ncores = ncores
        assert n % ncores == 0
        self.nl = n // ncores                  # dst nodes per core
        self.nbins = math.ceil(self.nl / P)    # bins per core
        self.nlp = self.nbins * P              # padded local nodes
        self.ntab = self.nlp * ncores          # rows in gathered tables
        assert fin % P == 0
        self.kt = fin // P                     # K-tiles for dense1


# ----------------------------------------------------------------------------
# Host-side graph preprocessing
# ----------------------------------------------------------------------------

def prep_graph(cfg: Cfg, edge_index: np.ndarray):
    """Bin-pack dsts, order edges, build index arrays for all cores.

    Returns dict with per-core arrays and the global slot permutation.
    """
    n, ncores, nl, nbins, nlp = cfg.n, cfg.ncores, cfg.nl, cfg.nbins, cfg.nlp
    src = np.concatenate([edge_index[0], np.arange(n, dtype=np.int64)])
    dst = np.concatenate([edge_index[1], np.arange(n, dtype=np.int64)])
    core = dst // nl

    # --- per-core bin-packing of dst nodes ---------------------------------
    slot_global = np.full(n, -1, dtype=np.int64)  # node -> row in table space
    bin_fill_cnt = np.zeros((ncores, nbins), dtype=np.int64)   # nodes per bin
    bin_fill_sum = np.zeros((ncores, nbins), dtype=np.int64)   # edges per bin
    deg_all = np.bincount(dst, minlength=n)
    import heapq
    for c in range(ncores):
        lo, hi = c * nl, (c + 1) * nl
        deg = deg_all[lo:hi]
        order = np.argsort(-deg, kind="stable")
        # heap of (edgesum, count, bin)
        heap = [(0, 0, b) for b in range(nbins)]
        heapq.heapify(heap)
        stash = []
        for node in order:
            d = int(deg[node])
            while True:
                s, cnt, b = heapq.heappop(heap)
                if cnt < P:
                    break
                stash.append((s, cnt, b))
            slot_global[lo + node] = c * nlp + b * P + cnt
            heapq.heappush(heap, (s + d, cnt + 1, b))
            for it in stash:
                heapq.heappush(heap, it)
            stash.clear()
        for s, cnt, b in heap:
            bin_fill_cnt[c, b], bin_fill_sum[c, b] = cnt, s

    T = int(math.ceil(bin_fill_sum.max() / P))
    T = max(T, 1)

    # --- order edges by (core, bin, slot) ----------------------------------
    dslot = slot_global[dst]              # global slot of dst
    order = np.argsort(dslot, kind="stable")
    src_o = src[order]
    dslot_o = dslot[order]
    # per (core, bin) counts in sorted order
    binid_o = dslot_o // P                # global bin id = core*nbins + bin
    nbins_tot = ncores * nbins
    cnts = np.bincount(binid_o, minlength=nbins_tot)
    offs = np.concatenate([[0], np.cumsum(cnts)])

    # padded edge arrays, [ncores, nbins, T*P]
    srcidx = np.zeros((ncores, nbins, T * P), dtype=np.int32)
    dstidx = np.zeros((ncores, nbins, T * P), dtype=np.int32)
    dstcol = np.full((ncores, nbins, T * P), 200.0, dtype=np.float32)
    for g in range(nbins_tot):
        c, b = divmod(g, nbins)
        s, cnt = offs[g], cnts[g]
        assert cnt <= T * P, f"bin overflow {cnt} > {T * P}"
        srcidx[c, b, :cnt] = slot_global[src_o[s:s + cnt]]
        loc = (dslot_o[s:s + cnt] - (c * nlp + b * P)).astype(np.int64)
        dstidx[c, b, :cnt] = b * P + loc
        dstcol[c, b, :cnt] = loc.astype(np.float32)

    # lane-major layout: edge j of a bin -> (tile t=j//P, lane p=j%P)
    # device wants [bin, lane(P), tile(T)]
    def to_dev(a):
        return np.ascontiguousarray(
            a.reshape(ncores, nbins, T, P).transpose(0, 1, 3, 2))

    # node permutation per core: slot s -> original node (or -1)
    perm = np.full((ncores, nlp), -1, dtype=np.int64)
    nodes = np.where(slot_global >= 0)[0]
    perm.reshape(-1)[slot_global[nodes]] = nodes

    return dict(
        T=T, slot_global=slot_global, perm=perm,
        srcidx=to_dev(srcidx), dstidx=to_dev(dstidx), dstcol=to_dev(dstcol),
    )


# ----------------------------------------------------------------------------
# Device program builders (single SPMD program, data differs per core)
# ----------------------------------------------------------------------------

def _new_nc(cfg):
    return bacc.Bacc("TRN2", target_bir_lowering=False, debug=False,
                     enable_asserts=False, num_devices=cfg.ncores)


def build_dense1(cfg: Cfg):
    """xT [fin, nlp] -> XL1 [nlp, hid+1] (zero col), XR1 [nlp, hid+1] (ones)."""
    nc = _new_nc(cfg)
    fin, hid, nlp, kt = cfg.fin, cfg.hid, cfg.nlp, cfg.kt
    xT = nc.dram_tensor("xT", [fin, nlp], F32, kind="ExternalInput")
    wl = nc.dram_tensor("wl", [fin, hid + 1], F32, kind="ExternalInput")
    wr = nc.dram_tensor("wr", [fin, hid + 1], F32, kind="ExternalInput")
    blB = nc.dram_tensor("blB", [P, hid + 1], F32, kind="ExternalInput")
    brB = nc.dram_tensor("brB", [P, hid + 1], F32, kind="ExternalInput")
    XL = nc.dram_tensor("XL1", [nlp, hid + 1], F32, kind="ExternalOutput")
    XR = nc.dram_tensor("XR1", [nlp, hid + 1], F32, kind="ExternalOutput")

    mtiles = nlp // P
    with tile.TileContext(nc) as tc:
        with tc.tile_pool(name="const", bufs=1) as cp, \
             tc.tile_pool(name="work", bufs=4) as wp, \
             tc.tile_pool(name="psum", bufs=4, space="PSUM") as pp:
            xk = cp.tile([P, kt, nlp], F32)
            nc.sync.dma_start(xk[:], xT[:].rearrange("(k p) n -> p k n", p=P))
            wl_sb = cp.tile([P, kt, hid + 1], F32)
            nc.sync.dma_start(wl_sb[:], wl[:].rearrange("(k p) h -> p k h", p=P))
            wr_sb = cp.tile([P, kt, hid + 1], F32)
            nc.sync.dma_start(wr_sb[:], wr[:].rearrange("(k p) h -> p k h", p=P))
            blB_sb = cp.tile([P, hid + 1], F32)
            nc.sync.dma_start(blB_sb[:], blB[:])
            brB_sb = cp.tile([P, hid + 1], F32)
            nc.sync.dma_start(brB_sb[:], brB[:])

            for m in range(mtiles):
                ms = slice(m * P, (m + 1) * P)
                psl = pp.tile([P, hid + 1], F32, tag="psl")
                psr = pp.tile([P, hid + 1], F32, tag="psr")
                for k in range(kt):
                    nc.tensor.matmul(psl[:], lhsT=xk[:, k, ms], rhs=wl_sb[:, k, :],
                                     start=(k == 0), stop=(k == kt - 1))
                for k in range(kt):
                    nc.tensor.matmul(psr[:], lhsT=xk[:, k, ms], rhs=wr_sb[:, k, :],
                                     start=(k == 0), stop=(k == kt - 1))
                ol = wp.tile([P, hid + 1], F32, tag="ol")
                nc.vector.tensor_tensor(out=ol[:], in0=psl[:], in1=blB_sb[:], op=ALU.add)
                orr = wp.tile([P, hid + 1], F32, tag="orr")
                nc.vector.tensor_tensor(out=orr[:], in0=psr[:], in1=brB_sb[:], op=ALU.add)
                nc.sync.dma_start(XL[ms, :], ol[:])
                nc.sync.dma_start(XR[ms, :], orr[:])
    nc.compile()
    return nc


def _edge_phase(nc, tc, cfg, T, pools, tabs, consts, n_feat, n_lay, feat_w,
                finalize):
    """Shared edge-pipeline over bins.

    n_feat: gathered row width (no ones col in XL table; XR has ones cols)
    n_lay:  number of layers packed in the row (1 or 2)
    feat_w: per-layer feature width (+1 for the ones col) -> row layout is
            n_lay blocks of (feat_w-1 feats + 1 ones col) = n_feat cols.
    finalize(b, psums): consume accumulated PSUM tiles for bin b.
    """
    cp, gp, wp, pp = pools
    XLfull, XR, srcidx, dstidx, dstcol = tabs
    iotaB_sb, attB_sb = consts
    for b in range(cfg.nbins):
        sidx = gp.tile([P, T], I32, tag="sidx")
        nc.sync.dma_start(sidx[:], srcidx[b])
        didx = gp.tile([P, T], I32, tag="didx")
        nc.sync.dma_start(didx[:], dstidx[b])
        dcol = gp.tile([P, T], F32, tag="dcol")
        nc.sync.dma_start(dcol[:], dstcol[b])

        # XL tables carry 1 in the ones-column positions (denominator source
        # for the scatter matmul whose rhs is G); the z-add also touches those
        # columns but their att weight is 0, so they never affect the logits.
        # walrus only lowers one-index-per-partition indirect DMAs correctly,
        # so gather tile-by-tile.
        G = gp.tile([P, T, n_feat], F32, tag="G")
        R = gp.tile([P, T, n_feat], F32, tag="R")
        for t in range(T):
            nc.gpsimd.indirect_dma_start(
                out=G[:, t, :], out_offset=None, in_=XLfull[:],
                in_offset=IndirectOffsetOnAxis(ap=sidx[:, t:t + 1], axis=0))
            nc.gpsimd.indirect_dma_start(
                out=R[:, t, :], out_offset=None, in_=XR[:],
                in_offset=IndirectOffsetOnAxis(ap=didx[:, t:t + 1], axis=0))
        nc.vector.tensor_tensor(out=R[:], in0=R[:], in1=G[:], op=ALU.add)

        # leaky_relu(z) = max(0.2*z, z); Lrelu isn't in the ACT sim, use DVE
        U = wp.tile([P, T, n_feat], F32, tag="U")
        nc.vector.scalar_tensor_tensor(
            out=U[:], in0=R[:], scalar=SLOPE, in1=R[:],
            op0=ALU.mult, op1=ALU.max)
        nc.vector.tensor_tensor(out=U[:], in0=U[:], in1=attB_sb[:], op=ALU.mult)
        e = wp.tile([P, T, n_lay], F32, tag="e")
        nc.vector.tensor_reduce(
            out=e[:], in_=U[:].rearrange("p t (l f) -> p t l f", l=n_lay),
            axis=mybir.AxisListType.X, op=ALU.add)
        ee = wp.tile([P, T, n_lay], F32, tag="ee")
        nc.scalar.activation(out=ee[:], in_=e[:], func=AF.Exp)

        psums = [pp.tile([P, feat_w], F32, tag=f"ps{l}", name=f"ps{l}")
                 for l in range(n_lay)]
        for t in range(T):
            for l in range(n_lay):
                A = wp.tile([P, P], F32, tag="A")
                nc.vector.tensor_scalar(
                    out=A[:], in0=iotaB_sb[:],
                    scalar1=dcol[:, t:t + 1], scalar2=ee[:, t, l:l + 1],
                    op0=ALU.is_equal, op1=ALU.mult)
                nc.tensor.matmul(
                    psums[l][:], lhsT=A[:],
                    rhs=G[:, t, l * feat_w:(l + 1) * feat_w],
                    start=(t == 0), stop=(t == T - 1))
        finalize(b, psums)


def build_edge1(cfg: Cfg, T: int):
    """Edge phase of layer 1 + dense transforms of layers 2/3.

    Inputs: XL1full [ntab, hid], XR1 [nlp, hid+1], idx arrays, consts,
            w23l/w23r [hid, 2*(out+1)], biases.
    Outputs: XL23 [nlp, 2*(out+1)] (xl2|0|xl3|0), XR23 (xr2|1|xr3|1).
    """
    nc = _new_nc(cfg)
    hid, out, nlp, nbins = cfg.hid, cfg.out, cfg.nlp, cfg.nbins
    w23 = 2 * (out + 1)
    XLfull = nc.dram_tensor("XL1full", [cfg.ntab, hid + 1], F32, kind="ExternalInput")
    XR = nc.dram_tensor("XR1", [nlp, hid + 1], F32, kind="ExternalInput")
    srcidx = nc.dram_tensor("srcidx", [nbins, P, T], I32, kind="ExternalInput")
    dstidx = nc.dram_tensor("dstidx", [nbins, P, T], I32, kind="ExternalInput")
    dstcol = nc.dram_tensor("dstcol", [nbins, P, T], F32, kind="ExternalInput")
    iotaB = nc.dram_tensor("iotaB", [P, P], F32, kind="ExternalInput")
    attB = nc.dram_tensor("attB", [P, T, hid + 1], F32, kind="ExternalInput")
    b1B = nc.dram_tensor("b1B", [P, hid], F32, kind="ExternalInput")
    identB = nc.dram_tensor("identB", [P, P], F32, kind="ExternalInput")
    w23l = nc.dram_tensor("w23l", [hid, w23], F32, kind="ExternalInput")
    w23r = nc.dram_tensor("w23r", [hid, w23], F32, kind="ExternalInput")
    b23lB = nc.dram_tensor("b23lB", [P, w23], F32, kind="ExternalInput")
    b23rB = nc.dram_tensor("b23rB", [P, w23], F32, kind="ExternalInput")
    XL23 = nc.dram_tensor("XL23", [nlp, w23], F32, kind="ExternalOutput")
    XR23 = nc.dram_tensor("XR23", [nlp, w23], F32, kind="ExternalOutput")

    with tile.TileContext(nc) as tc:
        with tc.tile_pool(name="const", bufs=1) as cp, \
             tc.tile_pool(name="gath", bufs=3) as gp, \
             tc.tile_pool(name="work", bufs=3) as wp, \
             tc.tile_pool(name="psum", bufs=2, space="PSUM") as pp, \
             tc.tile_pool(name="psfin", bufs=2, space="PSUM") as pf:
            iotaB_sb = cp.tile([P, P], F32)
            nc.sync.dma_start(iotaB_sb[:], iotaB[:])
            attB_sb = cp.tile([P, T, hid + 1], F32)
            nc.sync.dma_start(attB_sb[:], attB[:])
            b1B_sb = cp.tile([P, hid], F32)
            nc.sync.dma_start(b1B_sb[:], b1B[:])
            ident_sb = cp.tile([P, P], F32)
            nc.sync.dma_start(ident_sb[:], identB[:])
            w23l_sb = cp.tile([hid, w23], F32)
            nc.sync.dma_start(w23l_sb[:], w23l[:])
            w23r_sb = cp.tile([hid, w23], F32)
            nc.sync.dma_start(w23r_sb[:], w23r[:])
            b23l_sb = cp.tile([P, w23], F32)
            nc.sync.dma_start(b23l_sb[:], b23lB[:])
            b23r_sb = cp.tile([P, w23], F32)
            nc.sync.dma_start(b23r_sb[:], b23rB[:])

            def finalize(b, psums):
                ps = psums[0]
                ms = slice(b * P, (b + 1) * P)
                d = wp.tile([P, 1], F32, tag="d")
                nc.vector.tensor_scalar_add(d[:], ps[:, hid:hid + 1], EPS)
                r = wp.tile([P, 1], F32, tag="r")
                nc.vector.reciprocal(r[:], d[:])
                h = wp.tile([P, hid], F32, tag="h")
                nc.vector.tensor_scalar(out=h[:], in0=ps[:, 0:hid], scalar1=r[:],
                                        scalar2=None, op0=ALU.mult)
                nc.vector.tensor_tensor(out=h[:], in0=h[:], in1=b1B_sb[:], op=ALU.add)
                nc.scalar.activation(out=h[:], in_=h[:], func=AF.Relu)
                # transpose h -> hT for the dense23 matmuls
                pst = pf.tile([P, P], F32, tag="pst")
                nc.tensor.transpose(out=pst[:], in_=h[:], identity=ident_sb[:])
                hT = wp.tile([P, P], F32, tag="hT")
                nc.vector.tensor_copy(hT[:], pst[:])
                psl = pf.tile([P, w23], F32, tag="psl")
                nc.tensor.matmul(psl[:], lhsT=hT[:, 0:hid], rhs=w23l_sb[:],
                                 start=True, stop=True)
                psr = pf.tile([P, w23], F32, tag="psr")
                nc.tensor.matmul(psr[:], lhsT=hT[:, 0:hid], rhs=w23r_sb[:],
                                 start=True, stop=True)
                ol = wp.tile([P, w23], F32, tag="ol")
                nc.vector.tensor_tensor(out=ol[:], in0=psl[:], in1=b23l_sb[:], op=ALU.add)
                orr = wp.tile([P, w23], F32, tag="orr")
                nc.vector.tensor_tensor(out=orr[:], in0=psr[:], in1=b23r_sb[:], op=ALU.add)
                nc.sync.dma_start(XL23[ms, :], ol[:])
                nc.sync.dma_start(XR23[ms, :], orr[:])

            _edge_phase(nc, tc, cfg, T, (cp, gp, wp, pp),
                        (XLfull, XR, srcidx, dstidx, dstcol),
                        (iotaB_sb, attB_sb), hid + 1, 1, hid + 1, finalize)
    nc.compile()
    return nc


def build_edge23(cfg: Cfg, T: int):
    """Edge phases of layers 2 and 3 (shared gather)."""
    nc = _new_nc(cfg)
    out, nlp, nbins = cfg.out, cfg.nlp, cfg.nbins
    w23 = 2 * (out + 1)
    XLfull = nc.dram_tensor("XL23full", [cfg.ntab, w23], F32, kind="ExternalInput")
    XR = nc.dram_tensor("XR23", [nlp, w23], F32, kind="ExternalInput")
    srcidx = nc.dram_tensor("srcidx", [nbins, P, T], I32, kind="ExternalInput")
    dstidx = nc.dram_tensor("dstidx", [nbins, P, T], I32, kind="ExternalInput")
    dstcol = nc.dram_tensor("dstcol", [nbins, P, T], F32, kind="ExternalInput")
    iotaB = nc.dram_tensor("iotaB", [P, P], F32, kind="ExternalInput")
    attB = nc.dram_tensor("attB", [P, T, w23], F32, kind="ExternalInput")
    bmuB = nc.dram_tensor("bmuB", [P, out], F32, kind="ExternalInput")
    blvB = nc.dram_tensor("blvB", [P, out], F32, kind="ExternalInput")
    MU = nc.dram_tensor("MU", [nlp, out], F32, kind="ExternalOutput")
    LV = nc.dram_tensor("LV", [nlp, out], F32, kind="ExternalOutput")

    with tile.TileContext(nc) as tc:
        with tc.tile_pool(name="const", bufs=1) as cp, \
             tc.tile_pool(name="gath", bufs=3) as gp, \
             tc.tile_pool(name="work", bufs=3) as wp, \
             tc.tile_pool(name="psum", bufs=2, space="PSUM") as pp:
            iotaB_sb = cp.tile([P, P], F32)
            nc.sync.dma_start(iotaB_sb[:], iotaB[:])
            attB_sb = cp.tile([P, T, w23], F32)
            nc.sync.dma_start(attB_sb[:], attB[:])
            bmu_sb = cp.tile([P, out], F32)
            nc.sync.dma_start(bmu_sb[:], bmuB[:])
            blv_sb = cp.tile([P, out], F32)
            nc.sync.dma_start(blv_sb[:], blvB[:])

            def finalize(b, psums):
                ms = slice(b * P, (b + 1) * P)
                for ps, bias, dest, tg in ((psums[0], bmu_sb, MU, "mu"),
                                           (psums[1], blv_sb, LV, "lv")):
                    d = wp.tile([P, 1], F32, tag=f"d{tg}")
                    nc.vector.tensor_scalar_add(d[:], ps[:, out:out + 1], EPS)
                    r = wp.tile([P, 1], F32, tag=f"r{tg}")
                    nc.vector.reciprocal(r[:], d[:])
                    o = wp.tile([P, out], F32, tag=f"o{tg}")
                    nc.vector.tensor_scalar(out=o[:], in0=ps[:, 0:out], scalar1=r[:],
                                            scalar2=None, op0=ALU.mult)
                    nc.vector.tensor_tensor(out=o[:], in0=o[:], in1=bias[:], op=ALU.add)
                    nc.sync.dma_start(dest[ms, :], o[:])

            _edge_phase(nc, tc, cfg, T, (cp, gp, wp, pp),
                        (XLfull, XR, srcidx, dstidx, dstcol),
                        (iotaB_sb, attB_sb), w23, 2, out + 1, finalize)
    nc.compile()
    return nc


# ----------------------------------------------------------------------------
# Host orchestration
# ----------------------------------------------------------------------------

def _bb(v, rows=P):
    """Broadcast a 1-D row vector to [rows, len] f32."""
    v = np.asarray(v, np.float32).reshape(1, -1)
    return np.ascontiguousarray(np.broadcast_to(v, (rows, v.shape[1])))


def _hw_runner(nc, in_maps, cfg, trace=False):
    from concourse import bass_utils
    r = bass_utils.run_bass_kernel_spmd(
        nc, in_maps, core_ids=list(range(cfg.ncores)), trace=trace)
    return r.results, r.exec_time_ns


class _State:
    """Cached compiled programs + prep, keyed by edge structure."""
    key = None
    progs = None
    prep = None


def build_progs(cfg, T):
    return dict(
        dense1=build_dense1(cfg),
        edge1=build_edge1(cfg, T),
        edge23=build_edge23(cfg, T),
    )


def forward(cfg, x, ei_unused, w, pr, progs, runner):
    T = pr["T"]
    perm = pr["perm"]                    # [ncores, nlp] node ids or -1
    profile = {}

    hid, out, nlp, ntab = cfg.hid, cfg.out, cfg.nlp, cfg.ntab
    w23 = 2 * (out + 1)

    # ---- launch A: dense1 --------------------------------------------------
    zcol = np.zeros((cfg.fin, 1), np.float32)
    wl_pad = np.concatenate([w["sh_Wl"], zcol], 1)
    wr_pad = np.concatenate([w["sh_Wr"], zcol], 1)
    blB = _bb(np.concatenate([w["sh_bl"], [1.0]]))   # ones col in XL1
    brB = _bb(np.concatenate([w["sh_br"], [0.0]]))
    in_maps = []
    for c in range(cfg.ncores):
        xs = np.zeros((nlp, cfg.fin), np.float32)
        sel = perm[c] >= 0
        xs[sel] = x[perm[c][sel]]
        in_maps.append(dict(
            xT=np.ascontiguousarray(xs.T), wl=wl_pad, wr=wr_pad,
            blB=blB, brB=brB))
    rA, profile["A"] = runner(progs["dense1"], in_maps, cfg)
    XL1full = np.concatenate([rA[c]["XL1"] for c in range(cfg.ncores)])
    XR1 = [rA[c]["XR1"] for c in range(cfg.ncores)]

    # ---- launch B: edge1 + dense23 ----------------------------------------
    # iota along free dim, same row [0..P-1] on every partition
    iotaB = _bb(np.arange(P, dtype=np.float32))
    att1 = w["sh_att"].reshape(-1)
    att1B = np.zeros((T, hid + 1), np.float32)
    att1B[:, 0:hid] = att1
    att1B = np.ascontiguousarray(
        np.broadcast_to(att1B, (P, T, hid + 1)))
    w23l = np.zeros((hid, w23), np.float32)
    w23l[:, 0:out] = w["mu_Wl"]
    w23l[:, out + 1:2 * out + 1] = w["lv_Wl"]
    w23r = np.zeros((hid, w23), np.float32)
    w23r[:, 0:out] = w["mu_Wr"]
    w23r[:, out + 1:2 * out + 1] = w["lv_Wr"]
    b23l = np.zeros(w23, np.float32)
    b23l[0:out] = w["mu_bl"]
    b23l[out] = 1.0                                   # ones cols in XL23
    b23l[out + 1:2 * out + 1] = w["lv_bl"]
    b23l[2 * out + 1] = 1.0
    b23r = np.zeros(w23, np.float32)
    b23r[0:out] = w["mu_br"]
    b23r[out + 1:2 * out + 1] = w["lv_br"]
    ident = np.eye(P, dtype=np.float32)
    in_maps = []
    for c in range(cfg.ncores):
        in_maps.append(dict(
            XL1full=XL1full, XR1=XR1[c],
            srcidx=pr["srcidx"][c], dstidx=pr["dstidx"][c], dstcol=pr["dstcol"][c],
            iotaB=iotaB, attB=att1B, b1B=_bb(w["sh_b"]), identB=ident,
            w23l=w23l, w23r=w23r, b23lB=_bb(b23l), b23rB=_bb(b23r)))
    rB, profile["B"] = runner(progs["edge1"], in_maps, cfg)
    XL23full = np.concatenate([rB[c]["XL23"] for c in range(cfg.ncores)])
    XR23 = [rB[c]["XR23"] for c in range(cfg.ncores)]

    # ---- launch C: edge23 --------------------------------------------------
    att23 = np.zeros((T, w23), np.float32)
    att23[:, 0:out] = w["mu_att"].reshape(-1)
    att23[:, out + 1:2 * out + 1] = w["lv_att"].reshape(-1)
    att23B = np.ascontiguousarray(np.broadcast_to(att23, (P, T, w23)))
    in_maps = []
    for c in range(cfg.ncores):
        in_maps.append(dict(
            XL23full=XL23full, XR23=XR23[c],
            srcidx=pr["srcidx"][c], dstidx=pr["dstidx"][c], dstcol=pr["dstcol"][c],
            iotaB=iotaB, attB=att23B,
            bmuB=_bb(w["mu_b"]), blvB=_bb(w["lv_b"])))
    rC, profile["C"] = runner(progs["edge23"], in_maps, cfg)

    MU = np.concatenate([rC[c]["MU"] for c in range(cfg.ncores)])
    LV = np.concatenate([rC[c]["LV"] for c in range(cfg.ncores)])
    mu = MU[pr["slot_global"]]
    lv = LV[pr["slot_global"]]
    return (mu, lv), profile


def kernel(**inputs):
    cfg = Cfg()
    x = np.asarray(inputs["x"], np.float32)
    ei = np.asarray(inputs["edge_index"]).astype(np.int64)
    w = {k: np.asarray(v, np.float32) for k, v in inputs.items()
         if k not in ("x", "edge_index")}

    key = hash(ei.tobytes())
    if _State.key != key:
        pr = prep_graph(cfg, ei)
        _State.prep = pr
        _State.progs = build_progs(cfg, pr["T"])
        _State.key = key

    trace = bool(int(os.environ.get("GAT_TRACE", "0")))
    runner = functools.partial(_hw_runner_traced, trace=trace)
    (mu, lv), profile = forward(cfg, x, ei, w, _State.prep, _State.progs, runner)
    kernel._last_profile = profile
    return (mu, lv)


def _hw_runner_traced(nc, in_maps, cfg, trace=False):
    return _hw_runner(nc, in_maps, cfg, trace=trace)


kernel._last_profile = None
